# revision 2
# baseline (speedup 1.0000x reference)
"""Trainium2 Bass kernel for nn_CustomLoss (argmax-distance weighted loss).

reference:
    arg = argmax(target, axis=1)              # [B]
    delta = distance[arg]                     # [B]
    err = |distance[None,:] - delta[:,None]| + 1
    loss = sum((output - target) * err) / B

Algorithm (bucket matmul, data-parallel over 8 NeuronCores):
  err[b,:] is one of 5 constant rows W[a,:] = 1 + |dist - dist[a]|, selected
  by a = argmax(target[b]).  So with the one-hot E[b,a] = [argmax==a]:
      loss*B = sum_{a,c} W[a,c] * P[a,c],   P = E^T @ (O - T)   (5x5)
  P is accumulated on TensorE: rows land on partitions (128) x free dim
  (g,c); per 16-row block l, matmul(lhsT=E_blk[128,80], rhs=[T|O]_blk
  [128,2,80]) accumulates out[(l,a),(s,l',c)] in PSUM over the whole shard;
  only the l==l' diagonal 5x5 blocks are meaningful and the host sums them.

Per-core work: DMA t (f32, HWDGE) + o (f32->bf16 cast, SWDGE); ScalarE casts
t->bf16; VectorE does max-reduce + is_ge (the only [B,C]-sized DVE passes);
TensorE runs 256 accumulating matmuls.  Output per core: [80, 160] f32.
"""

from contextlib import ExitStack

import numpy as np

P = 128
C = 5
DIST = (-0.5, -0.34, 0.0, 0.34, 0.5)
B = 4194304
NCORES = 8
ROWS_PER_CORE = B // NCORES  # 524288
G = 1024                     # rows per partition per tile
NTILES = ROWS_PER_CORE // (P * G)  # 4
FREE = G * C                 # 5120
BLK = 16                     # rows-per-partition per matmul block
BLKC = BLK * C               # 80 = lhsT columns = psum partitions
NBLK = G // BLK              # 64 matmul blocks per tile
MOUT = BLKC                  # 80
NOUT = 2 * BLKC              # 160

_CACHE = {}


def _build_nc():
    import concourse.bacc as bacc
    import concourse.mybir as mybir
    import concourse.tile as tile

    F32 = mybir.dt.float32
    BF16 = mybir.dt.bfloat16

    nc = bacc.Bacc(target_bir_lowering=False)

    t_in = nc.declare_dram_parameter("t", [ROWS_PER_CORE, C], F32, isOutput=False)
    o_in = nc.declare_dram_parameter("o", [ROWS_PER_CORE, C], F32, isOutput=False)
    out = nc.declare_dram_parameter("out", [MOUT, NOUT], F32, isOutput=True)

    # row = n*(P*G) + p*G + g ; per-partition data is contiguous in DRAM
    t_tiled = t_in.rearrange("(n p g) c -> n p (g c)", p=P, g=G)
    o_tiled = o_in.rearrange("(n p g) c -> n p (g c)", p=P, g=G)

    with ExitStack() as ctx:
        tc = ctx.enter_context(tile.TileContext(nc))
        pool = ctx.enter_context(tc.tile_pool(name="work", bufs=3))
        psp = ctx.enter_context(tc.tile_pool(name="ps", bufs=1, space="PSUM"))
        outp = ctx.enter_context(tc.tile_pool(name="outp", bufs=1))
        ps = psp.tile([MOUT, NOUT], F32)

        for k in range(NTILES):
            tt = pool.tile([P, FREE], F32, tag="t", name="tt", bufs=3)
            nc.sync.dma_start(tt[:, :], t_tiled[k])
            # to = [t_bf16 | o_bf16] side by side so one matmul streams both
            to = pool.tile([P, 2 * FREE], BF16, tag="to", name="to", bufs=3)
            nc.gpsimd.dma_start(to[:, FREE : 2 * FREE], o_tiled[k])  # DMA cast
            nc.scalar.copy(to[:, 0:FREE], tt[:, :])  # ACT cast f32->bf16

            tv = tt[:, :].rearrange("p (g c) -> p g c", c=C)
            m = pool.tile([P, G], F32, tag="m", name="m", bufs=3)
            nc.vector.tensor_reduce(
                m[:, :], tv, axis=mybir.AxisListType.X, op=mybir.AluOpType.max
            )
            E = pool.tile([P, FREE], BF16, tag="E", name="E", bufs=3)
            nc.vector.tensor_tensor(
                E[:, :].rearrange("p (g c) -> p g c", c=C),
                tv,
                m[:, :].to_broadcast([P, G, C]),
                op=mybir.AluOpType.is_ge,
            )

            tov = to[:, :].rearrange("p (s f) -> p s f", s=2)
            for blk in range(NBLK):
                first = k == 0 and blk == 0
                last = k == NTILES - 1 and blk == NBLK - 1
                sl = slice(blk * BLKC, (blk + 1) * BLKC)
                nc.tensor.matmul(
                    ps[:, :], E[:, sl], tov[:, :, sl], start=first, stop=last
                )

        res = outp.tile([MOUT, NOUT], F32)
        nc.scalar.copy(res[:, :], ps[:, :])
        nc.sync.dma_start(out[:, :], res[:, :])
    nc.finalize()
    return nc


def _get_nc():
    if "nc" not in _CACHE:
        _CACHE["nc"] = _build_nc()
    return _CACHE["nc"]


def _reduce_loss(results):
    """results: iterable of per-core out arrays [80, 160] f32 -> loss."""
    dist = np.asarray(DIST, np.float64)
    W = 1.0 + np.abs(dist[None, :] - dist[:, None])  # [a, c]
    total = 0.0
    for arr in results:
        r = arr.astype(np.float64).reshape(BLK, C, 2, BLK, C)  # (l,a,s,l',c)
        Pm = np.einsum("dasdc->sac", r)  # diag over l; [2(s=t,o), 5, 5]
        total += float((W * (Pm[1] - Pm[0])).sum())
    return total / B


def kernel(output, target, distance, _want_results=False):
    from concourse.bass_utils import run_bass_kernel_spmd

    output = np.asarray(output, dtype=np.float32)
    target = np.asarray(target, dtype=np.float32)
    distance = np.asarray(distance, dtype=np.float32)
    assert output.shape == (B, C) and target.shape == (B, C)
    assert np.allclose(distance, np.asarray(DIST, np.float32)), distance

    nc = _get_nc()
    o_sh = output.reshape(NCORES, ROWS_PER_CORE, C)
    t_sh = target.reshape(NCORES, ROWS_PER_CORE, C)
    in_maps = [
        {"t": np.ascontiguousarray(t_sh[i]), "o": np.ascontiguousarray(o_sh[i])}
        for i in range(NCORES)
    ]
    res = run_bass_kernel_spmd(nc, in_maps, core_ids=list(range(NCORES)))
    loss = np.float32(_reduce_loss(r["out"] for r in res.results))
    if _want_results:
        return loss, res
    return loss


# revision 3
# speedup vs baseline: 1.1569x; 1.1569x over previous
"""Trainium2 Bass kernel for nn_CustomLoss (argmax-distance weighted loss).

reference:
    arg = argmax(target, axis=1)              # [B]
    delta = distance[arg]                     # [B]
    err = |distance[None,:] - delta[:,None]| + 1
    loss = sum((output - target) * err) / B

Algorithm (bucket matmul, data-parallel over 8 NeuronCores):
  err[b,:] is one of 5 constant rows W[a,:] = 1 + |dist - dist[a]|, selected
  by a = argmax(target[b]).  So with the one-hot E[b,a] = [argmax==a]:
      loss*B = sum_{a,c} W[a,c] * P[a,c],   P = E^T @ (O - T)   (5x5)
  P is accumulated on TensorE: rows land on partitions (128) x free dim
  (g,c); per 16-row block l, matmul(lhsT=E_blk[128,80], rhs=[T|O]_blk
  [128,2,80]) accumulates out[(l,a),(s,l',c)] in PSUM over the whole shard;
  only the l==l' diagonal 5x5 blocks are meaningful and the host sums them.

Phase-ordered schedule: all 8 t tiles stream first (HWDGE/sync queue) and
feed the ScalarE cast + VectorE max/is_ge chains; the o loads (SWDGE cast
f32->bf16, gpsimd queue) are gated behind t tile 6 via a dummy GPSIMD read,
so the tail of the HBM stream feeds only cheap TensorE matmuls.  Output per
core: [80, 160] f32; host sums the block-diagonal 5x5s.
"""

from contextlib import ExitStack

import numpy as np

P = 128
C = 5
DIST = (-0.5, -0.34, 0.0, 0.34, 0.5)
B = 4194304
NCORES = 8
ROWS_PER_CORE = B // NCORES  # 524288
G = 512                      # rows per partition per tile
NTILES = ROWS_PER_CORE // (P * G)  # 8
FREE = G * C                 # 2560
HFREE = FREE // 2            # 1280 (o loads split in halves)
BLK = 16                     # rows-per-partition per matmul block
BLKC = BLK * C               # 80 = lhsT columns = psum partitions
NBLK = G // BLK              # 32 matmul blocks per tile
MOUT = BLKC                  # 80
NOUT = 2 * BLKC              # 160

_CACHE = {}


def _build_nc():
    import concourse.bacc as bacc
    import concourse.mybir as mybir
    import concourse.tile as tile

    F32 = mybir.dt.float32
    BF16 = mybir.dt.bfloat16

    nc = bacc.Bacc(target_bir_lowering=False)

    t_in = nc.declare_dram_parameter("t", [ROWS_PER_CORE, C], F32, isOutput=False)
    o_in = nc.declare_dram_parameter("o", [ROWS_PER_CORE, C], F32, isOutput=False)
    out = nc.declare_dram_parameter("out", [MOUT, NOUT], F32, isOutput=True)

    # row = n*(P*G) + p*G + g ; per-partition data is contiguous in DRAM
    t_tiled = t_in.rearrange("(n p g) c -> n p (g c)", p=P, g=G)
    o_tiled = o_in.rearrange("(n p g) c -> n p (g c)", p=P, g=G)

    with ExitStack() as ctx:
        tc = ctx.enter_context(tile.TileContext(nc))
        pool = ctx.enter_context(tc.tile_pool(name="work", bufs=2))
        psp = ctx.enter_context(tc.tile_pool(name="ps", bufs=1, space="PSUM"))
        outp = ctx.enter_context(tc.tile_pool(name="outp", bufs=1))
        ps = psp.tile([MOUT, NOUT], F32)

        # phase 1: the full t stream owns the DMA engines
        tts = []
        for k in range(NTILES):
            tt = pool.tile([P, FREE], F32, tag="t", name="tt", bufs=5)
            nc.sync.dma_start(tt[:, :], t_tiled[k])
            tts.append(tt)

        # to_k = [t_bf16 | o_bf16] side by side so one matmul streams both
        tos = [
            pool.tile([P, 2 * FREE], BF16, tag="to", name="to", bufs=NTILES)
            for _ in range(NTILES)
        ]

        # phase 2: gate the o stream (gpsimd queue) behind t tile 6 so the
        # HBM tail is o-only and feeds nothing but matmuls
        scr = pool.tile([1, 1], F32, tag="scr", name="scr", bufs=1)
        nc.gpsimd.tensor_tensor(
            scr[:, :], tts[6][0:1, 0:1], tts[6][0:1, 0:1], op=mybir.AluOpType.add
        )
        for k in range(NTILES):
            for h in range(2):
                nc.gpsimd.dma_start(  # f32 -> bf16 cast in DMA
                    tos[k][:, FREE + h * HFREE : FREE + (h + 1) * HFREE],
                    o_tiled[k][:, h * HFREE : (h + 1) * HFREE],
                )

        for k in range(NTILES):
            tt, to = tts[k], tos[k]
            nc.scalar.copy(to[:, 0:FREE], tt[:, :])  # ACT cast f32->bf16

            tv = tt[:, :].rearrange("p (g c) -> p g c", c=C)
            m = pool.tile([P, G], F32, tag="m", name="m", bufs=2)
            nc.vector.tensor_reduce(
                m[:, :], tv, axis=mybir.AxisListType.X, op=mybir.AluOpType.max
            )
            E = pool.tile([P, FREE], BF16, tag="E", name="E", bufs=NTILES)
            nc.vector.tensor_tensor(
                E[:, :].rearrange("p (g c) -> p g c", c=C),
                tv,
                m[:, :].to_broadcast([P, G, C]),
                op=mybir.AluOpType.is_ge,
            )

            tov = to[:, :].rearrange("p (s f) -> p s f", s=2)
            for blk in range(NBLK):
                first = k == 0 and blk == 0
                last = k == NTILES - 1 and blk == NBLK - 1
                sl = slice(blk * BLKC, (blk + 1) * BLKC)
                nc.tensor.matmul(
                    ps[:, :], E[:, sl], tov[:, :, sl], start=first, stop=last
                )

        res = outp.tile([MOUT, NOUT], F32)
        nc.scalar.copy(res[:, :], ps[:, :])
        nc.sync.dma_start(out[:, :], res[:, :])
    nc.finalize()
    return nc


def _get_nc():
    if "nc" not in _CACHE:
        _CACHE["nc"] = _build_nc()
    return _CACHE["nc"]


def _reduce_loss(results):
    """results: iterable of per-core out arrays [80, 160] f32 -> loss."""
    dist = np.asarray(DIST, np.float64)
    W = 1.0 + np.abs(dist[None, :] - dist[:, None])  # [a, c]
    total = 0.0
    for arr in results:
        r = arr.astype(np.float64).reshape(BLK, C, 2, BLK, C)  # (l,a,s,l',c)
        Pm = np.einsum("dasdc->sac", r)  # diag over l; [2(s=t,o), 5, 5]
        total += float((W * (Pm[1] - Pm[0])).sum())
    return total / B


def kernel(output, target, distance, _want_results=False):
    from concourse.bass_utils import run_bass_kernel_spmd

    output = np.asarray(output, dtype=np.float32)
    target = np.asarray(target, dtype=np.float32)
    distance = np.asarray(distance, dtype=np.float32)
    assert output.shape == (B, C) and target.shape == (B, C)
    assert np.allclose(distance, np.asarray(DIST, np.float32)), distance

    nc = _get_nc()
    o_sh = output.reshape(NCORES, ROWS_PER_CORE, C)
    t_sh = target.reshape(NCORES, ROWS_PER_CORE, C)
    in_maps = [
        {"t": np.ascontiguousarray(t_sh[i]), "o": np.ascontiguousarray(o_sh[i])}
        for i in range(NCORES)
    ]
    res = run_bass_kernel_spmd(nc, in_maps, core_ids=list(range(NCORES)))
    loss = np.float32(_reduce_loss(r["out"] for r in res.results))
    if _want_results:
        return loss, res
    return loss


# revision 4
# speedup vs baseline: 1.2925x; 1.1172x over previous
"""Trainium2 Bass kernel for nn_CustomLoss (argmax-distance weighted loss).

reference:
    arg = argmax(target, axis=1)              # [B]
    delta = distance[arg]                     # [B]
    err = |distance[None,:] - delta[:,None]| + 1
    loss = sum((output - target) * err) / B

Algorithm (bucket matmul, data-parallel over 8 NeuronCores):
  err[b,:] is one of 5 constant rows W[a,:] = 1 + |dist - dist[a]|, selected
  by a = argmax(target[b]).  So with the one-hot E[b,a] = [argmax==a]:
      loss*B = sum_{a,c} W[a,c] * P[a,c],   P = E^T @ (O - T)   (5x5)
  P is accumulated on TensorE: rows land on partitions (128) x free dim
  (g,c); per 16-row block l, matmul(lhsT=E_blk[128,80], rhs=[T|O]_blk
  [128,2,80]) accumulates out[(l,a),(s,l',c)] in PSUM over the whole shard;
  only the l==l' diagonal 5x5 blocks are meaningful and the host sums them.

Schedule: ALL loads go through the single sync HWDGE ring, whose FIFO order
enforces the phasing the Tile scheduler would otherwise break: 8 t tiles
first (they feed the ScalarE cast + VectorE max/is_ge chains), then the o
tiles in 16 half-DMAs, so the tail of the HBM stream feeds only cheap
TensorE matmuls.  No SWDGE involvement (avoids the DMA-engine-15 descriptor
ring contention); ScalarE casts both t and o to bf16.  max/is_ge are split
into tile halves so matmuls chase E at half-tile granularity.  Output per
core: [80, 160] f32; host sums the block-diagonal 5x5s.
"""

from contextlib import ExitStack

import numpy as np

P = 128
C = 5
DIST = (-0.5, -0.34, 0.0, 0.34, 0.5)
B = 4194304
NCORES = 8
ROWS_PER_CORE = B // NCORES  # 524288
G = 512                      # rows per partition per tile
NTILES = ROWS_PER_CORE // (P * G)  # 8
FREE = G * C                 # 2560
HG = G // 2                  # 256
HFREE = FREE // 2            # 1280
BLK = 16                     # rows-per-partition per matmul block
BLKC = BLK * C               # 80 = lhsT columns = psum partitions
NBLK = G // BLK              # 32 matmul blocks per tile
MOUT = BLKC                  # 80
NOUT = 2 * BLKC              # 160

_CACHE = {}


def _build_nc():
    import concourse.bacc as bacc
    import concourse.mybir as mybir
    import concourse.tile as tile

    F32 = mybir.dt.float32
    BF16 = mybir.dt.bfloat16

    nc = bacc.Bacc(target_bir_lowering=False)

    t_in = nc.declare_dram_parameter("t", [ROWS_PER_CORE, C], F32, isOutput=False)
    o_in = nc.declare_dram_parameter("o", [ROWS_PER_CORE, C], F32, isOutput=False)
    out = nc.declare_dram_parameter("out", [MOUT, NOUT], F32, isOutput=True)

    # row = n*(P*G) + p*G + g ; per-partition data is contiguous in DRAM
    t_tiled = t_in.rearrange("(n p g) c -> n p (g c)", p=P, g=G)
    o_tiled = o_in.rearrange("(n p g) c -> n p (g c)", p=P, g=G)

    with ExitStack() as ctx:
        tc = ctx.enter_context(tile.TileContext(nc))
        pool = ctx.enter_context(tc.tile_pool(name="work", bufs=2))
        psp = ctx.enter_context(tc.tile_pool(name="ps", bufs=1, space="PSUM"))
        outp = ctx.enter_context(tc.tile_pool(name="outp", bufs=1))
        ps = psp.tile([MOUT, NOUT], F32)

        # phase 1: the 8 t tiles head the HWDGE ring FIFO
        tts = []
        for k in range(NTILES):
            tt = pool.tile([P, FREE], F32, tag="t", name="tt", bufs=5)
            nc.sync.dma_start(tt[:, :], t_tiled[k])
            tts.append(tt)

        # phase 2: o tiles (f32, half-DMAs) queue behind them on the same ring
        ofs = []
        for k in range(NTILES):
            of = pool.tile([P, FREE], F32, tag="of", name="of", bufs=3)
            for h in range(2):
                nc.sync.dma_start(
                    of[:, h * HFREE : (h + 1) * HFREE],
                    o_tiled[k][:, h * HFREE : (h + 1) * HFREE],
                )
            ofs.append(of)

        # to_k = [t_bf16 | o_bf16] side by side so one matmul streams both
        tos = [
            pool.tile([P, 2 * FREE], BF16, tag="to", name="to", bufs=NTILES)
            for _ in range(NTILES)
        ]

        for k in range(NTILES):
            tt, of, to = tts[k], ofs[k], tos[k]
            nc.scalar.copy(to[:, 0:FREE], tt[:, :])  # ACT cast f32->bf16
            for h in range(2):  # ACT cast f32->bf16, chasing the o half-DMAs
                nc.scalar.copy(
                    to[:, FREE + h * HFREE : FREE + (h + 1) * HFREE],
                    of[:, h * HFREE : (h + 1) * HFREE],
                )

            E = pool.tile([P, FREE], BF16, tag="E", name="E", bufs=NTILES)
            for h in range(2):  # half-tile max/is_ge so matmuls chase E
                tv = tt[:, h * HFREE : (h + 1) * HFREE].rearrange(
                    "p (g c) -> p g c", c=C
                )
                m = pool.tile([P, HG], F32, tag="m", name="m", bufs=2)
                nc.vector.tensor_reduce(
                    m[:, :], tv, axis=mybir.AxisListType.X, op=mybir.AluOpType.max
                )
                nc.vector.tensor_tensor(
                    E[:, h * HFREE : (h + 1) * HFREE].rearrange(
                        "p (g c) -> p g c", c=C
                    ),
                    tv,
                    m[:, :].to_broadcast([P, HG, C]),
                    op=mybir.AluOpType.is_ge,
                )

            tov = to[:, :].rearrange("p (s f) -> p s f", s=2)
            for blk in range(NBLK):
                first = k == 0 and blk == 0
                last = k == NTILES - 1 and blk == NBLK - 1
                sl = slice(blk * BLKC, (blk + 1) * BLKC)
                nc.tensor.matmul(
                    ps[:, :], E[:, sl], tov[:, :, sl], start=first, stop=last
                )

        res = outp.tile([MOUT, NOUT], F32)
        nc.scalar.copy(res[:, :], ps[:, :])
        nc.sync.dma_start(out[:, :], res[:, :])
    nc.finalize()
    return nc


def _get_nc():
    if "nc" not in _CACHE:
        _CACHE["nc"] = _build_nc()
    return _CACHE["nc"]


def _reduce_loss(results):
    """results: iterable of per-core out arrays [80, 160] f32 -> loss."""
    dist = np.asarray(DIST, np.float64)
    W = 1.0 + np.abs(dist[None, :] - dist[:, None])  # [a, c]
    total = 0.0
    for arr in results:
        r = arr.astype(np.float64).reshape(BLK, C, 2, BLK, C)  # (l,a,s,l',c)
        Pm = np.einsum("dasdc->sac", r)  # diag over l; [2(s=t,o), 5, 5]
        total += float((W * (Pm[1] - Pm[0])).sum())
    return total / B


def kernel(output, target, distance, _want_results=False):
    from concourse.bass_utils import run_bass_kernel_spmd

    output = np.asarray(output, dtype=np.float32)
    target = np.asarray(target, dtype=np.float32)
    distance = np.asarray(distance, dtype=np.float32)
    assert output.shape == (B, C) and target.shape == (B, C)
    assert np.allclose(distance, np.asarray(DIST, np.float32)), distance

    nc = _get_nc()
    o_sh = output.reshape(NCORES, ROWS_PER_CORE, C)
    t_sh = target.reshape(NCORES, ROWS_PER_CORE, C)
    in_maps = [
        {"t": np.ascontiguousarray(t_sh[i]), "o": np.ascontiguousarray(o_sh[i])}
        for i in range(NCORES)
    ]
    res = run_bass_kernel_spmd(nc, in_maps, core_ids=list(range(NCORES)))
    loss = np.float32(_reduce_loss(r["out"] for r in res.results))
    if _want_results:
        return loss, res
    return loss


# revision 6
# speedup vs baseline: 1.3058x; 1.0103x over previous
"""Trainium2 Bass kernel for nn_CustomLoss (argmax-distance weighted loss).

reference:
    arg = argmax(target, axis=1)              # [B]
    delta = distance[arg]                     # [B]
    err = |distance[None,:] - delta[:,None]| + 1
    loss = sum((output - target) * err) / B

Algorithm (bucket matmul, data-parallel over 8 NeuronCores):
  err[b,:] is one of 5 constant rows W[a,:] = 1 + |dist - dist[a]|, selected
  by a = argmax(target[b]).  So with the one-hot E[b,a] = [argmax==a]:
      loss*B = sum_{a,c} W[a,c] * P[a,c],   P = E^T @ (O - T)   (5x5)
  P is accumulated on TensorE: rows land on partitions (128) x free dim
  (g,c); per 16-row block l, matmul(lhsT=E_blk[128,80], rhs=[T|O]_blk
  [128,2,80]) accumulates out[(l,a),(s,l',c)] in PSUM over the whole shard;
  only the l==l' diagonal 5x5 blocks are meaningful and the host sums them.

Schedule: ALL loads go through the single sync HWDGE ring, whose FIFO order
enforces the phasing the Tile scheduler would otherwise break: 8 t tiles
first (they feed the ScalarE cast + VectorE max/is_ge chains), then the o
tiles in 16 half-DMAs, so the tail of the HBM stream feeds only cheap
TensorE matmuls.  No SWDGE involvement (avoids the DMA-engine-15 descriptor
ring contention); ScalarE casts both t and o to bf16.  max/is_ge are split
into tile halves so matmuls chase E at half-tile granularity.  Output per
core: [80, 160] f32; host sums the block-diagonal 5x5s.
"""

from contextlib import ExitStack

import numpy as np

P = 128
C = 5
DIST = (-0.5, -0.34, 0.0, 0.34, 0.5)
B = 4194304
NCORES = 8
ROWS_PER_CORE = B // NCORES  # 524288
G = 512                      # rows per partition per tile
NTILES = ROWS_PER_CORE // (P * G)  # 8
FREE = G * C                 # 2560
HG = G // 2                  # 256
HFREE = FREE // 2            # 1280
BLK = 16                     # rows-per-partition per matmul block
BLKC = BLK * C               # 80 = lhsT columns = psum partitions
NBLK = G // BLK              # 32 matmul blocks per tile
MOUT = BLKC                  # 80
NOUT = 2 * BLKC              # 160

_CACHE = {}


def _build_nc():
    import concourse.bacc as bacc
    import concourse.mybir as mybir
    import concourse.tile as tile

    F32 = mybir.dt.float32
    BF16 = mybir.dt.bfloat16

    nc = bacc.Bacc(target_bir_lowering=False)

    t_in = nc.declare_dram_parameter("t", [ROWS_PER_CORE, C], F32, isOutput=False)
    o_in = nc.declare_dram_parameter("o", [ROWS_PER_CORE, C], F32, isOutput=False)
    out = nc.declare_dram_parameter("out", [MOUT, NOUT], F32, isOutput=True)

    # row = n*(P*G) + p*G + g ; per-partition data is contiguous in DRAM
    t_tiled = t_in.rearrange("(n p g) c -> n p (g c)", p=P, g=G)
    o_tiled = o_in.rearrange("(n p g) c -> n p (g c)", p=P, g=G)

    with ExitStack() as ctx:
        tc = ctx.enter_context(tile.TileContext(nc))
        pool = ctx.enter_context(tc.tile_pool(name="work", bufs=2))
        psp = ctx.enter_context(tc.tile_pool(name="ps", bufs=1, space="PSUM"))
        outp = ctx.enter_context(tc.tile_pool(name="outp", bufs=1))
        ps = psp.tile([MOUT, NOUT], F32)

        # phase 1: the 8 t tiles head the HWDGE ring FIFO
        tts = []
        for k in range(NTILES):
            tt = pool.tile([P, FREE], F32, tag="t", name="tt", bufs=5)
            nc.sync.dma_start(tt[:, :], t_tiled[k])
            tts.append(tt)

        # phase 2: o tiles (f32) queue behind them on the same ring; the last
        # tile lands in quarters so the tail chases at ~0.8us granularity
        ofs = []
        for k in range(NTILES):
            of = pool.tile([P, FREE], F32, tag="of", name="of", bufs=3)
            nq = 4 if k == NTILES - 1 else 2
            q = FREE // nq
            for h in range(nq):
                nc.sync.dma_start(
                    of[:, h * q : (h + 1) * q],
                    o_tiled[k][:, h * q : (h + 1) * q],
                )
            ofs.append(of)

        # to_k = [t_bf16 | o_bf16] side by side so one matmul streams both
        tos = [
            pool.tile([P, 2 * FREE], BF16, tag="to", name="to", bufs=NTILES)
            for _ in range(NTILES)
        ]

        for k in range(NTILES):
            tt, of, to = tts[k], ofs[k], tos[k]
            nc.scalar.copy(to[:, 0:FREE], tt[:, :])  # ACT cast f32->bf16
            if k < NTILES - 1:  # ACT cast f32->bf16, chasing the o half-DMAs
                for h in range(2):
                    nc.scalar.copy(
                        to[:, FREE + h * HFREE : FREE + (h + 1) * HFREE],
                        of[:, h * HFREE : (h + 1) * HFREE],
                    )
            else:
                # last tile: VectorE is free by now and ACT lags the stream;
                # cast the four o quarters on DVE (2x fp32 single-src mode)
                q = FREE // 4
                for h in range(4):
                    nc.vector.tensor_copy(
                        to[:, FREE + h * q : FREE + (h + 1) * q],
                        of[:, h * q : (h + 1) * q],
                    )

            E = pool.tile([P, FREE], BF16, tag="E", name="E", bufs=NTILES)
            for h in range(2):  # half-tile max/is_ge so matmuls chase E
                tv = tt[:, h * HFREE : (h + 1) * HFREE].rearrange(
                    "p (g c) -> p g c", c=C
                )
                m = pool.tile([P, HG], F32, tag="m", name="m", bufs=2)
                nc.vector.tensor_reduce(
                    m[:, :], tv, axis=mybir.AxisListType.X, op=mybir.AluOpType.max
                )
                nc.vector.tensor_tensor(
                    E[:, h * HFREE : (h + 1) * HFREE].rearrange(
                        "p (g c) -> p g c", c=C
                    ),
                    tv,
                    m[:, :].to_broadcast([P, HG, C]),
                    op=mybir.AluOpType.is_ge,
                )

            tov = to[:, :].rearrange("p (s f) -> p s f", s=2)
            for blk in range(NBLK):
                first = k == 0 and blk == 0
                last = k == NTILES - 1 and blk == NBLK - 1
                sl = slice(blk * BLKC, (blk + 1) * BLKC)
                nc.tensor.matmul(
                    ps[:, :], E[:, sl], tov[:, :, sl], start=first, stop=last
                )

        res = outp.tile([MOUT, NOUT], F32)
        nc.scalar.copy(res[:, :], ps[:, :])
        nc.sync.dma_start(out[:, :], res[:, :])
    nc.finalize()
    return nc


def _get_nc():
    if "nc" not in _CACHE:
        _CACHE["nc"] = _build_nc()
    return _CACHE["nc"]


def _reduce_loss(results):
    """results: iterable of per-core out arrays [80, 160] f32 -> loss."""
    dist = np.asarray(DIST, np.float64)
    W = 1.0 + np.abs(dist[None, :] - dist[:, None])  # [a, c]
    total = 0.0
    for arr in results:
        r = arr.astype(np.float64).reshape(BLK, C, 2, BLK, C)  # (l,a,s,l',c)
        Pm = np.einsum("dasdc->sac", r)  # diag over l; [2(s=t,o), 5, 5]
        total += float((W * (Pm[1] - Pm[0])).sum())
    return total / B


def kernel(output, target, distance, _want_results=False):
    from concourse.bass_utils import run_bass_kernel_spmd

    output = np.asarray(output, dtype=np.float32)
    target = np.asarray(target, dtype=np.float32)
    distance = np.asarray(distance, dtype=np.float32)
    assert output.shape == (B, C) and target.shape == (B, C)
    assert np.allclose(distance, np.asarray(DIST, np.float32)), distance

    nc = _get_nc()
    o_sh = output.reshape(NCORES, ROWS_PER_CORE, C)
    t_sh = target.reshape(NCORES, ROWS_PER_CORE, C)
    in_maps = [
        {"t": np.ascontiguousarray(t_sh[i]), "o": np.ascontiguousarray(o_sh[i])}
        for i in range(NCORES)
    ]
    res = run_bass_kernel_spmd(nc, in_maps, core_ids=list(range(NCORES)))
    loss = np.float32(_reduce_loss(r["out"] for r in res.results))
    if _want_results:
        return loss, res
    return loss
